# revision 2
# baseline (speedup 1.0000x reference)
"""Causal self-attention (B=2, T=2048, C=1024, H=16) on 8 Trainium2 NeuronCores.

Sharding: data-parallel over batch (2) x tensor-parallel over heads (4 groups
of 4 heads) = 8 cores. c_attn column-sharded, c_proj row-sharded; each core
emits a partial [C, T] projection output that the host sums per batch.

All matmuls run in bf16 with fp32 PSUM accumulation. Attention scores are
computed transposed (S^T = K Q^T, k on partitions). The PV matmul keeps V
stationary (65 columns: 64 V dims + a ones column that accumulates the
softmax denominator) and streams P 512 wide. The softmax normalization row
1/denom is broadcast across partitions with a gpsimd partition_broadcast
(keeping the PE free and saving a PSUM bank so the QKV/proj accumulators can
double-buffer). Input DMAs are ordered so the first matmul's operands land
first; output is stored bf16 to halve the write-back traffic.
"""

import numpy as np
import ml_dtypes

BF = ml_dtypes.bfloat16

B, T, C, H, DH = 2, 2048, 1024, 16, 64
N_CORES = 8
G = 4            # head groups (tensor-parallel)
HPG = 4          # heads per group
TQ = 512         # query strip width
TK = 128         # key tile width
NSTRIP = T // TQ        # 4 query strips
NKT = T // TK           # 16 key tiles
NCT = C // 128          # 8 contraction tiles for qkv
VST = 136               # V2 per-k-tile stride: 2 heads x (64 V + 1 ones + 3 pad)

_CACHE = {}


def _ensure_runtime():
    """Import jax (boots the axon PJRT plugin) exactly once."""
    import jax
    jax.devices()


def _build(with_bias: bool):
    import concourse.tile as tile
    from concourse import bacc, mybir

    f32 = mybir.dt.float32
    bf16 = mybir.dt.bfloat16
    Exp = mybir.ActivationFunctionType.Exp

    nc = bacc.Bacc("TRN2", target_bir_lowering=False, debug=False,
                   enable_asserts=False, num_devices=N_CORES)

    xT_d = nc.dram_tensor("xT", [C, T], bf16, kind="ExternalInput").ap()
    wqk_d = nc.dram_tensor("wqk", [C, 512], bf16, kind="ExternalInput").ap()
    wv_d = nc.dram_tensor("wv", [C, 256], bf16, kind="ExternalInput").ap()
    wp_d = nc.dram_tensor("wp", [256, C], bf16, kind="ExternalInput").ap()
    if with_bias:
        bqk_d = nc.dram_tensor("bqk", [1, 512], bf16, kind="ExternalInput").ap()
        bv_d = nc.dram_tensor("bv", [1, 256], bf16, kind="ExternalInput").ap()
    out_d = nc.dram_tensor("outT", [C, T], bf16, kind="ExternalOutput").ap()

    with tile.TileContext(nc) as tc:
        with (
            tc.tile_pool(name="persist", bufs=1) as pp,
            tc.tile_pool(name="pP", bufs=34) as pP,
            tc.tile_pool(name="rrow", bufs=6) as pRR,
            tc.tile_pool(name="bcsb", bufs=6) as pBC,
            tc.tile_pool(name="ob", bufs=4) as pO,
            tc.tile_pool(name="psum", bufs=1, space="PSUM") as psp,
        ):
            # ---- persistent SBUF tensors -------------------------------
            xT = [pp.tile([128, T], bf16, tag=f"xT{i}", name=f"xT{i}")
                  for i in range(NCT)]
            wqk = [pp.tile([128, 512], bf16, tag=f"wqk{i}", name=f"wqk{i}")
                   for i in range(NCT)]
            wv = [pp.tile([128, 256], bf16, tag=f"wv{i}", name=f"wv{i}")
                  for i in range(NCT)]
            wp = [pp.tile([128, C], bf16, tag=f"wp{p}", name=f"wp{p}")
                  for p in range(2)]
            QTp = [pp.tile([128, T], bf16, tag=f"QT{p}", name=f"QT{p}")
                   for p in range(2)]
            KTp = [pp.tile([128, T], bf16, tag=f"KT{p}", name=f"KT{p}")
                   for p in range(2)]
            # both pairs' V in one tile so a v_step needs one copy
            V2 = pp.tile([128, 2 * NKT * VST], bf16, tag="V2", name="V2")
            yT2 = [pp.tile([128, T], bf16, tag=f"yT{p}", name=f"yT{p}")
                   for p in range(2)]
            tri2 = pp.tile([128, 256], bf16, tag="tri2", name="tri2")
            if with_bias:
                bqk = pp.tile([1, 512], bf16, tag="bqk", name="bqk")
                bv = pp.tile([1, 256], bf16, tag="bv", name="bv")
                ones_row = pp.tile([1, 512], bf16, tag="ones", name="ones")

            # ---- input DMAs, ordered by first use ----------------------
            # pair-0 QK weights (cols 0:128 q01, 256:384 k01), then x strip 0
            for i in range(NCT):
                nc.sync.dma_start(wqk[i][:, 0:128], wqk_d[i * 128:(i + 1) * 128, 0:128])
                nc.sync.dma_start(wqk[i][:, 256:384], wqk_d[i * 128:(i + 1) * 128, 256:384])
            for i in range(NCT):
                nc.sync.dma_start(xT[i][:, 0:512], xT_d[i * 128:(i + 1) * 128, 0:512])
            for i in range(NCT):
                nc.sync.dma_start(xT[i][:, 512:1024], xT_d[i * 128:(i + 1) * 128, 512:1024])
            for i in range(NCT):
                nc.sync.dma_start(wqk[i][:, 128:256], wqk_d[i * 128:(i + 1) * 128, 128:256])
                nc.sync.dma_start(wqk[i][:, 384:512], wqk_d[i * 128:(i + 1) * 128, 384:512])
            for i in range(NCT):
                nc.sync.dma_start(wv[i][:], wv_d[i * 128:(i + 1) * 128, :])
            for c in range(2, 4):
                for i in range(NCT):
                    nc.sync.dma_start(
                        xT[i][:, c * 512:(c + 1) * 512],
                        xT_d[i * 128:(i + 1) * 128, c * 512:(c + 1) * 512])
            for p in range(2):
                nc.sync.dma_start(wp[p][:], wp_d[p * 128:(p + 1) * 128, :])

            # ---- constants ---------------------------------------------
            # tri2[i, 256] = two side-by-side copies of tri[i, j] = (i <= j)
            nc.gpsimd.memset(tri2[:], 1.0)
            for hh in range(2):
                nc.gpsimd.affine_select(
                    out=tri2[:, hh * 128:(hh + 1) * 128],
                    in_=tri2[:, hh * 128:(hh + 1) * 128],
                    compare_op=mybir.AluOpType.is_ge,
                    fill=0.0, base=0, pattern=[[1, 128]], channel_multiplier=-1)
            if with_bias:
                nc.sync.dma_start(bqk[:], bqk_d[:, :])
                nc.sync.dma_start(bv[:], bv_d[:, :])
                nc.gpsimd.memset(ones_row[:], 1.0)
            # ones columns in V2 (denominator accumulators), one strided memset
            v2v = V2[:].rearrange("q (p kt h e) -> q p kt h e", p=2, kt=NKT, h=2)
            nc.gpsimd.memset(v2v[:, :, :, :, 64:65], 1.0)

            nbias = 1 if with_bias else 0
            P_store = {}

            def A_steps(pair, qt):
                """Step list: each step emits one complete psum group."""
                steps = []

                def qk_step(mt):
                    def f():
                        ps = psp.tile([128, TQ], f32, tag="big", bufs=2, name="psA")
                        for ci in range(NCT):
                            nc.tensor.matmul(
                                ps[:],
                                lhsT=wqk[ci][:, mt * 128:(mt + 1) * 128],
                                rhs=xT[ci][:, qt * TQ:(qt + 1) * TQ],
                                start=(ci == 0), stop=(ci == NCT + nbias - 1))
                        if with_bias:
                            nc.tensor.matmul(
                                ps[:], lhsT=bqk[0:1, mt * 128:(mt + 1) * 128],
                                rhs=ones_row[0:1, 0:TQ], start=False, stop=True)
                        dst = QTp[pair] if mt < 2 else KTp[pair]
                        nc.vector.tensor_copy(dst[:, qt * TQ:(qt + 1) * TQ], ps[:])
                    return f

                def v_step(kt):
                    def f():
                        psv = psp.tile([128, 256], f32, tag="big", bufs=2, name="psVt")
                        for ci in range(NCT):
                            nc.tensor.matmul(
                                psv[:],
                                lhsT=xT[ci][:, kt * 128:(kt + 1) * 128],
                                rhs=wv[ci][:, :],
                                start=(ci == 0), stop=(ci == NCT + nbias - 1))
                        if with_bias:
                            nc.tensor.matmul(
                                psv[:], lhsT=ones_row[0:1, 0:128], rhs=bv[0:1, :],
                                start=False, stop=True)
                        # [128, 2p, 2h, 64] -> strided V2 slots in one copy
                        s4 = psv[:].rearrange("q (p h b) -> q p h b", p=2, h=2)
                        d4 = v2v[:, :, kt, :, 0:64]
                        nc.vector.tensor_copy(d4, s4)
                    return f

                for mt in (pair, 2 + pair):
                    steps.append(qk_step(mt))
                if pair == 1:
                    for kt in range(4 * qt, 4 * qt + 4):
                        steps.append(v_step(kt))
                return steps

            def S_steps(pair, qt):
                """One step per k-tile: paired QK matmuls (disjoint PE row
                groups) + one exp."""
                nk = 4 * (qt + 1)

                def kt_step(kt):
                    def f():
                        ps = psp.tile([128, 2 * TQ], f32, tag="S", bufs=2, name="psS")
                        m = kt - 4 * qt
                        off = max(0, m) * 128
                        for hh in range(2):
                            nc.tensor.matmul(
                                ps[:, hh * TQ + off:(hh + 1) * TQ],
                                lhsT=KTp[pair][hh * 64:(hh + 1) * 64,
                                               kt * 128:(kt + 1) * 128],
                                rhs=QTp[pair][hh * 64:(hh + 1) * 64,
                                              qt * TQ + off:(qt + 1) * TQ],
                                start=True, stop=True)
                        Pt = pP.tile([128, 2 * TQ], bf16, tag="P", bufs=32, name="Pt")
                        if m < 0:    # one exp across both heads' banks
                            nc.scalar.activation(Pt[:, :], ps[:, :], Exp, scale=0.125)
                        else:        # one strided exp covering both heads
                            pse = ps[:].rearrange("p (h w) -> p h w", h=2)[:, :, off:TQ]
                            pte = Pt[:].rearrange("p (h w) -> p h w", h=2)[:, :, off:TQ]
                            nc.scalar.activation(pte, pse, Exp, scale=0.125)
                            # diagonal block: keep k <= q only, both heads at once
                            ptm = Pt[:].rearrange("p (h w) -> p h w", h=2)[:, :, off:off + 128]
                            nc.vector.tensor_mul(
                                ptm, ptm,
                                tri2[:].rearrange("p (h w) -> p h w", h=2))
                        for hh in range(2):
                            h = 2 * pair + hh
                            P_store[(h, qt, kt)] = Pt[:, hh * TQ:(hh + 1) * TQ]
                    return f

                return [kt_step(kt) for kt in range(nk)]

            def PV_steps(pair, qt):
                """One step per head: PV accumulation (V stationary) plus the
                partition-broadcast normalization straight into yT2."""
                nk = 4 * (qt + 1)

                def head_step(hh):
                    def f():
                        h = 2 * pair + hh
                        psy = psp.tile([65, TQ], f32, tag="pv", bufs=2, name="psy")
                        for kt in range(nk):
                            off = max(0, kt - 4 * qt) * 128
                            nc.tensor.matmul(
                                psy[:, off:TQ],
                                lhsT=v2v[:, pair, kt, hh, 0:65],
                                rhs=P_store[(h, qt, kt)][:, off:TQ],
                                start=(kt == 0), stop=(kt == nk - 1))
                        rrow = pRR.tile([1, TQ], bf16, tag="rrb", name="rrow")
                        with nc.allow_low_precision("softmax recip in bf16"):
                            nc.vector.reciprocal(rrow[0:1, :], psy[64:65, :])
                        bcs = pBC.tile([64, TQ], bf16, tag="bc", name="bcs")
                        nc.gpsimd.partition_broadcast(bcs[:], rrow[0:1, :])
                        nc.vector.tensor_mul(
                            yT2[pair][hh * 64:(hh + 1) * 64, qt * TQ:(qt + 1) * TQ],
                            psy[0:64, :], bcs[:])
                        if hh == 1:
                            for kt in range(nk):
                                del P_store[(2 * pair, qt, kt)]
                                del P_store[(2 * pair + 1, qt, kt)]
                    return f

                return [head_step(0), head_step(1)]

            def PJ_steps(qt):
                def co_step(co):
                    def f():
                        pso = psp.tile([128, TQ], f32, tag="big", bufs=2, name="psO")
                        for pair in range(2):
                            nc.tensor.matmul(
                                pso[:],
                                lhsT=wp[pair][:, co * 128:(co + 1) * 128],
                                rhs=yT2[pair][:, qt * TQ:(qt + 1) * TQ],
                                start=(pair == 0), stop=(pair == 1))
                        ob = pO.tile([128, TQ], bf16, tag="ob", name="ob")
                        nc.any.tensor_copy(ob[:], pso[:])
                        nc.sync.dma_start(
                            out_d[co * 128:(co + 1) * 128, qt * TQ:(qt + 1) * TQ],
                            ob[:])
                    return f

                return [co_step(co) for co in range(8)]

            def weave(s_list, others):
                """Interleave `others` proportionally between S k-tile steps."""
                if not s_list:
                    for f in others:
                        f()
                    return
                r = len(others) / len(s_list)
                acc, oi = 0.5, 0
                for f in s_list:
                    f()
                    acc += r
                    while acc >= 1.0 and oi < len(others):
                        others[oi]()
                        oi += 1
                        acc -= 1.0
                while oi < len(others):
                    others[oi]()
                    oi += 1

            # ---- software-pipelined, finely woven emission order -------
            for f in A_steps(0, 0):
                f()
            weave(S_steps(0, 0), A_steps(0, 1))
            weave(S_steps(0, 1), A_steps(1, 0))
            weave(S_steps(1, 0), PV_steps(0, 0) + A_steps(0, 2))
            weave(S_steps(0, 2), PV_steps(1, 0) + A_steps(1, 1) + PJ_steps(0))
            weave(S_steps(1, 1), PV_steps(0, 1) + A_steps(0, 3) + A_steps(1, 2))
            weave(S_steps(0, 3), PV_steps(1, 1) + A_steps(1, 3) + PJ_steps(1))
            weave(S_steps(1, 2), PV_steps(0, 2) + PV_steps(0, 3))
            weave(S_steps(1, 3), PV_steps(1, 2) + PJ_steps(2))
            weave([], PV_steps(1, 3) + PJ_steps(3))

    nc.compile()
    return nc


def _get_nc(with_bias: bool):
    key = ("nc", with_bias)
    if key not in _CACHE:
        _ensure_runtime()
        _CACHE[key] = _build(with_bias)
    return _CACHE[key]


def _shard_inputs(x, w_qkv, b_qkv, w_proj, with_bias):
    """Build the 8 per-core input maps (bf16)."""
    in_maps = []
    for core in range(N_CORES):
        b, g = core // G, core % G
        hs = [g * HPG + i for i in range(HPG)]
        q_cols = [w_qkv[:, h * DH:(h + 1) * DH] for h in hs]
        k_cols = [w_qkv[:, C + h * DH: C + (h + 1) * DH] for h in hs]
        v_cols = [w_qkv[:, 2 * C + h * DH: 2 * C + (h + 1) * DH] for h in hs]
        m = {
            "xT": np.ascontiguousarray(x[b].T).astype(BF),
            "wqk": np.concatenate(q_cols + k_cols, axis=1).astype(BF),
            "wv": np.concatenate(v_cols, axis=1).astype(BF),
            "wp": np.concatenate(
                [w_proj[h * DH:(h + 1) * DH, :] for h in hs], axis=0).astype(BF),
        }
        if with_bias:
            bq = [b_qkv[h * DH:(h + 1) * DH] for h in hs]
            bk = [b_qkv[C + h * DH: C + (h + 1) * DH] for h in hs]
            bvs = [b_qkv[2 * C + h * DH: 2 * C + (h + 1) * DH] for h in hs]
            m["bqk"] = np.concatenate(bq + bk)[None, :].astype(BF)
            m["bv"] = np.concatenate(bvs)[None, :].astype(BF)
        in_maps.append(m)
    return in_maps


def run_on_device(x, w_qkv, b_qkv, w_proj, b_proj, trace=False, trace_kwargs=None):
    """Returns (output [B,T,C] float32, BassKernelResults)."""
    x = np.asarray(x, np.float32)
    w_qkv = np.asarray(w_qkv, np.float32)
    b_qkv = np.asarray(b_qkv, np.float32)
    w_proj = np.asarray(w_proj, np.float32)
    b_proj = np.asarray(b_proj, np.float32)

    with_bias = bool(np.any(b_qkv))
    nc = _get_nc(with_bias)
    in_maps = _shard_inputs(x, w_qkv, b_qkv, w_proj, with_bias)

    from concourse.bass_utils import run_bass_kernel_spmd
    res = run_bass_kernel_spmd(nc, in_maps, core_ids=list(range(N_CORES)),
                               trace=trace, **(trace_kwargs or {}))

    out = np.zeros((B, T, C), np.float64)
    for core in range(N_CORES):
        b = core // G
        out[b] += res.results[core]["outT"].T.astype(np.float64)
    out += b_proj.astype(np.float64)[None, None, :]
    return out.astype(np.float32), res


def kernel(x, w_qkv, b_qkv, w_proj, b_proj):
    out, _ = run_on_device(x, w_qkv, b_qkv, w_proj, b_proj)
    return out


# revision 3
# speedup vs baseline: 1.2815x; 1.2815x over previous
"""Causal self-attention (B=2, T=2048, C=1024, H=16) on 8 Trainium2 NeuronCores.

Sharding: data-parallel over batch (2) x tensor-parallel over heads (4 groups
of 4 heads) = 8 cores. c_attn column-sharded, c_proj row-sharded; each core
emits a partial [C, T] bf16 projection output that the host sums per batch.

All matmuls run in bf16 with fp32 PSUM accumulation. Attention scores are
computed transposed (S^T = K Q^T, k on partitions). The PV matmul keeps V
stationary (65 columns: 64 V dims + a ones column that accumulates the
softmax denominator) and streams P 512 wide. The softmax normalization row
1/denom is broadcast across partitions with a gpsimd partition_broadcast
(keeping the PE free and saving a PSUM bank so the QKV/proj accumulators can
double-buffer). x/wqk/wv live in single SBUF tiles so one DMA instruction
covers all 8 contraction tiles (few large DMAs, issued on both HWDGE queues:
sync + scalar), which collapses the DMA prologue before the first matmul.
"""

import numpy as np
import ml_dtypes

BF = ml_dtypes.bfloat16

B, T, C, H, DH = 2, 2048, 1024, 16, 64
N_CORES = 8
G = 4            # head groups (tensor-parallel)
HPG = 4          # heads per group
TQ = 512         # query strip width
TK = 128         # key tile width
NSTRIP = T // TQ        # 4 query strips
NKT = T // TK           # 16 key tiles
NCT = C // 128          # 8 contraction tiles for qkv
VST = 136               # V2 per-k-tile stride: 2 heads x (64 V + 1 ones + 3 pad)

_CACHE = {}


def _ensure_runtime():
    """Import jax (boots the axon PJRT plugin) exactly once."""
    import jax
    jax.devices()


def _build(with_bias: bool):
    import concourse.tile as tile
    from concourse import bacc, mybir

    f32 = mybir.dt.float32
    bf16 = mybir.dt.bfloat16
    Exp = mybir.ActivationFunctionType.Exp

    nc = bacc.Bacc("TRN2", target_bir_lowering=False, debug=False,
                   enable_asserts=False, num_devices=N_CORES)

    xT_d = nc.dram_tensor("xT", [C, T], bf16, kind="ExternalInput").ap()
    wqk_d = nc.dram_tensor("wqk", [C, 512], bf16, kind="ExternalInput").ap()
    wv_d = nc.dram_tensor("wv", [C, 256], bf16, kind="ExternalInput").ap()
    wp_d = nc.dram_tensor("wp", [256, C], bf16, kind="ExternalInput").ap()
    if with_bias:
        bqk_d = nc.dram_tensor("bqk", [1, 512], bf16, kind="ExternalInput").ap()
        bv_d = nc.dram_tensor("bv", [1, 256], bf16, kind="ExternalInput").ap()
    out_d = nc.dram_tensor("outT", [C, T], bf16, kind="ExternalOutput").ap()

    with tile.TileContext(nc) as tc:
        with (
            tc.tile_pool(name="persist", bufs=1) as pp,
            tc.tile_pool(name="pP", bufs=34) as pP,
            tc.tile_pool(name="rrow", bufs=6) as pRR,
            tc.tile_pool(name="bcsb", bufs=6) as pBC,
            tc.tile_pool(name="ob", bufs=4) as pO,
            tc.tile_pool(name="psum", bufs=1, space="PSUM") as psp,
        ):
            # ---- persistent SBUF tensors (single tiles => single DMAs) --
            xA = pp.tile([128, NCT * T], bf16, tag="xA", name="xA")
            wqkA = pp.tile([128, NCT * 512], bf16, tag="wqkA", name="wqkA")
            wvA = pp.tile([128, NCT * 256], bf16, tag="wvA", name="wvA")
            wp = pp.tile([128, 2 * C], bf16, tag="wp", name="wp")
            QTp = [pp.tile([128, T], bf16, tag=f"QT{p}", name=f"QT{p}")
                   for p in range(2)]
            KTp = [pp.tile([128, T], bf16, tag=f"KT{p}", name=f"KT{p}")
                   for p in range(2)]
            V2 = pp.tile([128, 2 * NKT * VST], bf16, tag="V2", name="V2")
            yT2 = [pp.tile([128, T], bf16, tag=f"yT{p}", name=f"yT{p}")
                   for p in range(2)]
            tri2 = pp.tile([128, 256], bf16, tag="tri2", name="tri2")
            ones64 = pp.tile([1, 64], bf16, tag="ones64", name="ones64")
            if with_bias:
                bqk = pp.tile([1, 512], bf16, tag="bqk", name="bqk")
                bv = pp.tile([1, 256], bf16, tag="bv", name="bv")
                ones_row = pp.tile([1, 512], bf16, tag="ones", name="ones")

            def xa(ci):
                return xA[:, ci * T:(ci + 1) * T]

            def wqka(ci):
                return wqkA[:, ci * 512:(ci + 1) * 512]

            def wva(ci):
                return wvA[:, ci * 256:(ci + 1) * 256]

            # ---- input DMAs: few, large, ordered by first use ----------
            # 3D views [part, ci, col] over the single tiles
            xA3 = xA[:].rearrange("p (ci t) -> p ci t", ci=NCT)
            wqkA3 = wqkA[:].rearrange("p (ci c) -> p ci c", ci=NCT)
            wvA3 = wvA[:].rearrange("p (ci c) -> p ci c", ci=NCT)
            xd3 = xT_d.rearrange("(ci p) t -> p ci t", p=128)
            wqkd3 = wqk_d.rearrange("(ci p) c -> p ci c", p=128)
            wvd3 = wv_d.rearrange("(ci p) c -> p ci c", p=128)

            # first qk steps need wqk mt blocks 0&2 + x strip 0
            nc.sync.dma_start(wqkA3[:, :, 0:128], wqkd3[:, :, 0:128])
            nc.scalar.dma_start(wqkA3[:, :, 256:384], wqkd3[:, :, 256:384])
            nc.sync.dma_start(xA3[:, 0:4, 0:512], xd3[:, 0:4, 0:512])
            nc.scalar.dma_start(xA3[:, 4:8, 0:512], xd3[:, 4:8, 0:512])
            # strip 1, pair-1 wqk blocks, V weights
            nc.sync.dma_start(xA3[:, 0:4, 512:1024], xd3[:, 0:4, 512:1024])
            nc.scalar.dma_start(xA3[:, 4:8, 512:1024], xd3[:, 4:8, 512:1024])
            nc.sync.dma_start(wqkA3[:, :, 128:256], wqkd3[:, :, 128:256])
            nc.scalar.dma_start(wqkA3[:, :, 384:512], wqkd3[:, :, 384:512])
            nc.sync.dma_start(wvA3[:, 0:4, :], wvd3[:, 0:4, :])
            nc.scalar.dma_start(wvA3[:, 4:8, :], wvd3[:, 4:8, :])
            # strips 2-3, then proj weights
            nc.sync.dma_start(xA3[:, 0:4, 1024:2048], xd3[:, 0:4, 1024:2048])
            nc.scalar.dma_start(xA3[:, 4:8, 1024:2048], xd3[:, 4:8, 1024:2048])
            nc.sync.dma_start(
                wp[:].rearrange("p (b c) -> p b c", b=2),
                wp_d.rearrange("(b p) c -> p b c", p=128))

            # ---- constants ---------------------------------------------
            # tri2[i, 256] = two side-by-side copies of tri[i, j] = (i <= j)
            nc.gpsimd.memset(tri2[:], 1.0)
            for hh in range(2):
                nc.gpsimd.affine_select(
                    out=tri2[:, hh * 128:(hh + 1) * 128],
                    in_=tri2[:, hh * 128:(hh + 1) * 128],
                    compare_op=mybir.AluOpType.is_ge,
                    fill=0.0, base=0, pattern=[[1, 128]], channel_multiplier=-1)
            nc.gpsimd.memset(ones64[:], 1.0)
            if with_bias:
                nc.sync.dma_start(bqk[:], bqk_d[:, :])
                nc.scalar.dma_start(bv[:], bv_d[:, :])
                nc.gpsimd.memset(ones_row[:], 1.0)
            # ones columns in V2 (denominator accumulators), one strided memset
            v2v = V2[:].rearrange("q (p kt h e) -> q p kt h e", p=2, kt=NKT, h=2)
            nc.gpsimd.memset(v2v[:, :, :, :, 64:65], 1.0)

            nbias = 1 if with_bias else 0
            P_store = {}

            def A_steps(pair, qt):
                """Step list: each step emits one complete psum group."""
                steps = []

                def qk_step(mt):
                    def f():
                        ps = psp.tile([128, TQ], f32, tag="big", bufs=2, name="psA")
                        for ci in range(NCT):
                            nc.tensor.matmul(
                                ps[:],
                                lhsT=wqka(ci)[:, mt * 128:(mt + 1) * 128],
                                rhs=xa(ci)[:, qt * TQ:(qt + 1) * TQ],
                                start=(ci == 0), stop=(ci == NCT + nbias - 1))
                        if with_bias:
                            nc.tensor.matmul(
                                ps[:], lhsT=bqk[0:1, mt * 128:(mt + 1) * 128],
                                rhs=ones_row[0:1, 0:TQ], start=False, stop=True)
                        dst = QTp[pair] if mt < 2 else KTp[pair]
                        nc.vector.tensor_copy(dst[:, qt * TQ:(qt + 1) * TQ], ps[:])
                    return f

                def v_step(kt):
                    def f():
                        psv = psp.tile([128, 256], f32, tag="big", bufs=2, name="psVt")
                        for ci in range(NCT):
                            nc.tensor.matmul(
                                psv[:],
                                lhsT=xa(ci)[:, kt * 128:(kt + 1) * 128],
                                rhs=wva(ci)[:, :],
                                start=(ci == 0), stop=(ci == NCT + nbias - 1))
                        if with_bias:
                            nc.tensor.matmul(
                                psv[:], lhsT=ones_row[0:1, 0:128], rhs=bv[0:1, :],
                                start=False, stop=True)
                        # [128, 2p, 2h, 64] -> strided V2 slots in one copy
                        s4 = psv[:].rearrange("q (p h b) -> q p h b", p=2, h=2)
                        d4 = v2v[:, :, kt, :, 0:64]
                        nc.vector.tensor_copy(d4, s4)
                    return f

                for mt in (pair, 2 + pair):
                    steps.append(qk_step(mt))
                if pair == 1:
                    for kt in range(4 * qt, 4 * qt + 4):
                        steps.append(v_step(kt))
                return steps

            def S_steps(pair, qt):
                """One step per k-tile: paired QK matmuls (disjoint PE row
                groups) + one exp."""
                nk = 4 * (qt + 1)

                def kt_step(kt):
                    def f():
                        ps = psp.tile([128, 2 * TQ], f32, tag="S", bufs=2, name="psS")
                        m = kt - 4 * qt
                        off = max(0, m) * 128
                        for hh in range(2):
                            nc.tensor.matmul(
                                ps[:, hh * TQ + off:(hh + 1) * TQ],
                                lhsT=KTp[pair][hh * 64:(hh + 1) * 64,
                                               kt * 128:(kt + 1) * 128],
                                rhs=QTp[pair][hh * 64:(hh + 1) * 64,
                                              qt * TQ + off:(qt + 1) * TQ],
                                start=True, stop=True)
                        Pt = pP.tile([128, 2 * TQ], bf16, tag="P", bufs=32, name="Pt")
                        if m < 0:    # one exp across both heads' banks
                            nc.scalar.activation(Pt[:, :], ps[:, :], Exp, scale=0.125)
                        else:        # one strided exp covering both heads
                            pse = ps[:].rearrange("p (h w) -> p h w", h=2)[:, :, off:TQ]
                            pte = Pt[:].rearrange("p (h w) -> p h w", h=2)[:, :, off:TQ]
                            nc.scalar.activation(pte, pse, Exp, scale=0.125)
                            # diagonal block: keep k <= q only, both heads at once
                            ptm = Pt[:].rearrange("p (h w) -> p h w", h=2)[:, :, off:off + 128]
                            nc.vector.tensor_mul(
                                ptm, ptm,
                                tri2[:].rearrange("p (h w) -> p h w", h=2))
                        for hh in range(2):
                            h = 2 * pair + hh
                            P_store[(h, qt, kt)] = Pt[:, hh * TQ:(hh + 1) * TQ]
                    return f

                return [kt_step(kt) for kt in range(nk)]

            def PV_steps(pair, qt):
                """One step per head: PV accumulation (V stationary) plus the
                partition-broadcast normalization straight into yT2."""
                nk = 4 * (qt + 1)

                def head_step(hh):
                    def f():
                        h = 2 * pair + hh
                        psy = psp.tile([65, TQ], f32, tag="pv", bufs=2, name="psy")
                        for kt in range(nk):
                            off = max(0, kt - 4 * qt) * 128
                            nc.tensor.matmul(
                                psy[:, off:TQ],
                                lhsT=v2v[:, pair, kt, hh, 0:65],
                                rhs=P_store[(h, qt, kt)][:, off:TQ],
                                start=(kt == 0), stop=(kt == nk - 1))
                        drow = pRR.tile([1, TQ], f32, tag="rr", name="drow")
                        nc.vector.tensor_copy(drow[:], psy[64:65, :])
                        rec4 = pRR.tile([128, 4], f32, tag="r4", name="rec4")
                        nc.sync.dma_start(rec4[:, :], drow[0:1, :])
                        rec4b = pRR.tile([128, 4], bf16, tag="r4b", name="rec4b")
                        with nc.allow_low_precision("softmax recip in bf16"):
                            nc.vector.reciprocal(rec4b[:], rec4[:])
                        rrow = pRR.tile([1, TQ], bf16, tag="rrb", name="rrow")
                        nc.sync.dma_start(rrow[0:1, :], rec4b[:, :])
                        bcs = pBC.tile([64, TQ], bf16, tag="bc", name="bcs")
                        nc.gpsimd.partition_broadcast(bcs[:], rrow[0:1, :])
                        nc.vector.tensor_mul(
                            yT2[pair][hh * 64:(hh + 1) * 64, qt * TQ:(qt + 1) * TQ],
                            psy[0:64, :], bcs[:])
                        if hh == 1:
                            for kt in range(nk):
                                del P_store[(2 * pair, qt, kt)]
                                del P_store[(2 * pair + 1, qt, kt)]
                    return f

                return [head_step(0), head_step(1)]

            def PJ_steps(qt):
                def co_step(co):
                    def f():
                        pso = psp.tile([128, TQ], f32, tag="big", bufs=2, name="psO")
                        for pair in range(2):
                            nc.tensor.matmul(
                                pso[:],
                                lhsT=wp[:, pair * C + co * 128:pair * C + (co + 1) * 128],
                                rhs=yT2[pair][:, qt * TQ:(qt + 1) * TQ],
                                start=(pair == 0), stop=(pair == 1))
                        ob = pO.tile([128, TQ], bf16, tag="ob", name="ob")
                        nc.any.tensor_copy(ob[:], pso[:])
                        nc.sync.dma_start(
                            out_d[co * 128:(co + 1) * 128, qt * TQ:(qt + 1) * TQ],
                            ob[:])
                    return f

                return [co_step(co) for co in range(8)]

            def weave(s_list, others):
                """Interleave `others` proportionally between S k-tile steps."""
                if not s_list:
                    for f in others:
                        f()
                    return
                r = len(others) / len(s_list)
                acc, oi = 0.5, 0
                for f in s_list:
                    f()
                    acc += r
                    while acc >= 1.0 and oi < len(others):
                        others[oi]()
                        oi += 1
                        acc -= 1.0
                while oi < len(others):
                    others[oi]()
                    oi += 1

            # ---- software-pipelined, finely woven emission order -------
            for f in A_steps(0, 0):
                f()
            weave(S_steps(0, 0), A_steps(0, 1))
            weave(S_steps(0, 1), A_steps(1, 0))
            weave(S_steps(1, 0), PV_steps(0, 0) + A_steps(0, 2))
            weave(S_steps(0, 2), PV_steps(1, 0) + A_steps(1, 1) + PJ_steps(0))
            weave(S_steps(1, 1), PV_steps(0, 1) + A_steps(0, 3) + A_steps(1, 2))
            weave(S_steps(0, 3), PV_steps(1, 1) + A_steps(1, 3) + PJ_steps(1))
            weave(S_steps(1, 2), PV_steps(0, 2) + PV_steps(0, 3))
            weave(S_steps(1, 3), PV_steps(1, 2) + PJ_steps(2))
            weave([], PV_steps(1, 3) + PJ_steps(3))

    nc.compile()
    return nc


def _get_nc(with_bias: bool):
    key = ("nc", with_bias)
    if key not in _CACHE:
        _ensure_runtime()
        _CACHE[key] = _build(with_bias)
    return _CACHE[key]


def _shard_inputs(x, w_qkv, b_qkv, w_proj, with_bias):
    """Build the 8 per-core input maps (bf16)."""
    in_maps = []
    for core in range(N_CORES):
        b, g = core // G, core % G
        hs = [g * HPG + i for i in range(HPG)]
        q_cols = [w_qkv[:, h * DH:(h + 1) * DH] for h in hs]
        k_cols = [w_qkv[:, C + h * DH: C + (h + 1) * DH] for h in hs]
        v_cols = [w_qkv[:, 2 * C + h * DH: 2 * C + (h + 1) * DH] for h in hs]
        m = {
            "xT": np.ascontiguousarray(x[b].T).astype(BF),
            "wqk": np.concatenate(q_cols + k_cols, axis=1).astype(BF),
            "wv": np.concatenate(v_cols, axis=1).astype(BF),
            "wp": np.concatenate(
                [w_proj[h * DH:(h + 1) * DH, :] for h in hs], axis=0).astype(BF),
        }
        if with_bias:
            bq = [b_qkv[h * DH:(h + 1) * DH] for h in hs]
            bk = [b_qkv[C + h * DH: C + (h + 1) * DH] for h in hs]
            bvs = [b_qkv[2 * C + h * DH: 2 * C + (h + 1) * DH] for h in hs]
            m["bqk"] = np.concatenate(bq + bk)[None, :].astype(BF)
            m["bv"] = np.concatenate(bvs)[None, :].astype(BF)
        in_maps.append(m)
    return in_maps


def run_on_device(x, w_qkv, b_qkv, w_proj, b_proj, trace=False, trace_kwargs=None):
    """Returns (output [B,T,C] float32, BassKernelResults)."""
    x = np.asarray(x, np.float32)
    w_qkv = np.asarray(w_qkv, np.float32)
    b_qkv = np.asarray(b_qkv, np.float32)
    w_proj = np.asarray(w_proj, np.float32)
    b_proj = np.asarray(b_proj, np.float32)

    with_bias = bool(np.any(b_qkv))
    nc = _get_nc(with_bias)
    in_maps = _shard_inputs(x, w_qkv, b_qkv, w_proj, with_bias)

    from concourse.bass_utils import run_bass_kernel_spmd
    res = run_bass_kernel_spmd(nc, in_maps, core_ids=list(range(N_CORES)),
                               trace=trace, **(trace_kwargs or {}))

    out = np.zeros((B, T, C), np.float64)
    for core in range(N_CORES):
        b = core // G
        out[b] += res.results[core]["outT"].T.astype(np.float64)
    out += b_proj.astype(np.float64)[None, None, :]
    return out.astype(np.float32), res


def kernel(x, w_qkv, b_qkv, w_proj, b_proj):
    out, _ = run_on_device(x, w_qkv, b_qkv, w_proj, b_proj)
    return out


# revision 4
# speedup vs baseline: 1.2939x; 1.0097x over previous
"""Causal self-attention (B=2, T=2048, C=1024, H=16) on 8 Trainium2 NeuronCores.

Sharding: data-parallel over batch (2) x tensor-parallel over heads (4 groups
of 4 heads) = 8 cores. c_attn column-sharded, c_proj row-sharded; each core
emits a partial [C, T] bf16 projection output that the host sums per batch.

All matmuls run in bf16 with fp32 PSUM accumulation. Attention scores are
computed transposed (S^T = K Q^T, k on partitions). The PV matmul keeps V
stationary (65 columns: 64 V dims + a ones column that accumulates the
softmax denominator) and streams P 512 wide. The softmax normalization row
1/denom is broadcast across partitions with a gpsimd partition_broadcast.

DMA strategy: the host repacks x (strip-major), wqk (mt-block-major), wv and
wp so that every input lands with one large DMA of 2-8KB contiguous lines
(DMA engines are packet-rate-bound below ~2KB), split across both HWDGE
queues (sync + scalar), ordered by first use. Dummy warm-up matmuls run
during the DMA prologue so the PE HAM clock-gate is already released when
real work starts. The final projection is split per c_proj half so its
pair-0 matmuls overlap the last softmax-normalization chain.
"""

import numpy as np
import ml_dtypes

BF = ml_dtypes.bfloat16

B, T, C, H, DH = 2, 2048, 1024, 16, 64
N_CORES = 8
G = 4            # head groups (tensor-parallel)
HPG = 4          # heads per group
TQ = 512         # query strip width
TK = 128         # key tile width
NSTRIP = T // TQ        # 4 query strips
NKT = T // TK           # 16 key tiles
NCT = C // 128          # 8 contraction tiles for qkv
VST = 136               # V2 per-k-tile stride: 2 heads x (64 V + 1 ones + 3 pad)

_CACHE = {}


def _ensure_runtime():
    """Import jax (boots the axon PJRT plugin) exactly once."""
    import jax
    jax.devices()


def _build(with_bias: bool):
    import concourse.tile as tile
    from concourse import bacc, mybir

    f32 = mybir.dt.float32
    bf16 = mybir.dt.bfloat16
    Exp = mybir.ActivationFunctionType.Exp

    nc = bacc.Bacc("TRN2", target_bir_lowering=False, debug=False,
                   enable_asserts=False, num_devices=N_CORES)

    # host-repacked layouts (see _shard_inputs):
    #   xT:  [4 strips * 128 p, 8 ci * 512 col]   (8KB lines)
    #   wqk: [4 mt * 128 p, 8 ci * 128 col]       (2KB lines)
    #   wv:  [128 p, 8 ci * 256 col]              (4KB lines)
    #   wp:  [128 p, 2 pair * 1024 col]           (4KB lines)
    xT_d = nc.dram_tensor("xT", [NSTRIP * 128, NCT * TQ], bf16,
                          kind="ExternalInput").ap()
    wqk_d = nc.dram_tensor("wqk", [4 * 128, NCT * 128], bf16,
                           kind="ExternalInput").ap()
    wv_d = nc.dram_tensor("wv", [128, NCT * 256], bf16, kind="ExternalInput").ap()
    wp_d = nc.dram_tensor("wp", [128, 2 * C], bf16, kind="ExternalInput").ap()
    if with_bias:
        bqk_d = nc.dram_tensor("bqk", [1, 512], bf16, kind="ExternalInput").ap()
        bv_d = nc.dram_tensor("bv", [1, 256], bf16, kind="ExternalInput").ap()
    out_d = nc.dram_tensor("outT", [C, T], bf16, kind="ExternalOutput").ap()

    with tile.TileContext(nc) as tc:
        with (
            tc.tile_pool(name="persist", bufs=1) as pp,
            tc.tile_pool(name="pP", bufs=34) as pP,
            tc.tile_pool(name="rrow", bufs=6) as pRR,
            tc.tile_pool(name="bcsb", bufs=6) as pBC,
            tc.tile_pool(name="ob", bufs=4) as pO,
            tc.tile_pool(name="psum", bufs=1, space="PSUM") as psp,
        ):
            # ---- persistent SBUF tensors (single tiles => single DMAs) --
            # xA strip-major: [p, strip, ci, col]
            xA = pp.tile([128, NSTRIP * NCT * TQ], bf16, tag="xA", name="xA")
            # wqkA mt-major: [p, mt, ci, col]
            wqkA = pp.tile([128, 4 * NCT * 128], bf16, tag="wqkA", name="wqkA")
            wvA = pp.tile([128, NCT * 256], bf16, tag="wvA", name="wvA")
            wp = pp.tile([128, 2 * C], bf16, tag="wp", name="wp")
            QTp = [pp.tile([128, T], bf16, tag=f"QT{p}", name=f"QT{p}")
                   for p in range(2)]
            KTp = [pp.tile([128, T], bf16, tag=f"KT{p}", name=f"KT{p}")
                   for p in range(2)]
            V2 = pp.tile([128, 2 * NKT * VST], bf16, tag="V2", name="V2")
            yT2 = [pp.tile([128, T], bf16, tag=f"yT{p}", name=f"yT{p}")
                   for p in range(2)]
            tri2 = pp.tile([128, 256], bf16, tag="tri2", name="tri2")
            if with_bias:
                bqk = pp.tile([1, 512], bf16, tag="bqk", name="bqk")
                bv = pp.tile([1, 256], bf16, tag="bv", name="bv")
                ones_row = pp.tile([1, 512], bf16, tag="ones", name="ones")

            def xa(ci, strip):
                base = (strip * NCT + ci) * TQ
                return xA[:, base:base + TQ]

            def wqka(ci, mt):
                base = (mt * NCT + ci) * 128
                return wqkA[:, base:base + 128]

            def wva(ci):
                return wvA[:, ci * 256:(ci + 1) * 256]

            # ---- input DMAs: few, large lines, ordered by first use ----
            def x_strip_dma(eng, strip):
                eng.dma_start(
                    xA[:, strip * NCT * TQ:(strip + 1) * NCT * TQ],
                    xT_d[strip * 128:(strip + 1) * 128, :])

            def wqk_mt_dma(eng, mt):
                eng.dma_start(
                    wqkA[:, mt * NCT * 128:(mt + 1) * NCT * 128],
                    wqk_d[mt * 128:(mt + 1) * 128, :])

            # first qk steps need x strip 0 + wqk mt blocks 0 & 2
            x_strip_dma(nc.sync, 0)
            wqk_mt_dma(nc.scalar, 0)
            wqk_mt_dma(nc.sync, 2)
            x_strip_dma(nc.scalar, 1)
            wqk_mt_dma(nc.sync, 1)
            wqk_mt_dma(nc.scalar, 3)
            nc.sync.dma_start(wvA[:], wv_d[:, :])
            x_strip_dma(nc.scalar, 2)
            x_strip_dma(nc.sync, 3)
            nc.scalar.dma_start(wp[:], wp_d[:, :])

            # ---- constants ---------------------------------------------
            # tri2[i, 256] = two side-by-side copies of tri[i, j] = (i <= j)
            nc.gpsimd.memset(tri2[:], 1.0)
            for hh in range(2):
                nc.gpsimd.affine_select(
                    out=tri2[:, hh * 128:(hh + 1) * 128],
                    in_=tri2[:, hh * 128:(hh + 1) * 128],
                    compare_op=mybir.AluOpType.is_ge,
                    fill=0.0, base=0, pattern=[[1, 128]], channel_multiplier=-1)
            if with_bias:
                nc.sync.dma_start(bqk[:], bqk_d[:, :])
                nc.scalar.dma_start(bv[:], bv_d[:, :])
                nc.gpsimd.memset(ones_row[:], 1.0)
            # ones columns in V2 (denominator accumulators), one strided memset
            v2v = V2[:].rearrange("q (p kt h e) -> q p kt h e", p=2, kt=NKT, h=2)
            nc.gpsimd.memset(v2v[:, :, :, :, 64:65], 1.0)

            # ---- PE warm-up during the DMA prologue --------------------
            # ~20 matmuls on tri2 flip the HAM clock gate to 8/8 before the
            # first real matmul; results go to a scratch psum never read.
            warm = psp.tile([128, 256], f32, tag="S", bufs=2, name="warm")
            for _ in range(20):
                nc.tensor.matmul(warm[:], lhsT=tri2[:, 0:128], rhs=tri2[:],
                                 start=True, stop=True)

            nbias = 1 if with_bias else 0
            P_store = {}

            def A_steps(pair, qt):
                """Step list: each step emits one complete psum group."""
                steps = []

                def qk_step(mt):
                    def f():
                        ps = psp.tile([128, TQ], f32, tag="big", bufs=2, name="psA")
                        for ci in range(NCT):
                            nc.tensor.matmul(
                                ps[:],
                                lhsT=wqka(ci, mt),
                                rhs=xa(ci, qt),
                                start=(ci == 0), stop=(ci == NCT + nbias - 1))
                        if with_bias:
                            nc.tensor.matmul(
                                ps[:], lhsT=bqk[0:1, mt * 128:(mt + 1) * 128],
                                rhs=ones_row[0:1, 0:TQ], start=False, stop=True)
                        dst = QTp[pair] if mt < 2 else KTp[pair]
                        nc.vector.tensor_copy(dst[:, qt * TQ:(qt + 1) * TQ], ps[:])
                    return f

                def v_step(kt):
                    def f():
                        psv = psp.tile([128, 256], f32, tag="big", bufs=2, name="psVt")
                        strip, sub = kt // 4, kt % 4
                        for ci in range(NCT):
                            nc.tensor.matmul(
                                psv[:],
                                lhsT=xa(ci, strip)[:, sub * 128:(sub + 1) * 128],
                                rhs=wva(ci),
                                start=(ci == 0), stop=(ci == NCT + nbias - 1))
                        if with_bias:
                            nc.tensor.matmul(
                                psv[:], lhsT=ones_row[0:1, 0:128], rhs=bv[0:1, :],
                                start=False, stop=True)
                        # [128, 2p, 2h, 64] -> strided V2 slots in one copy
                        s4 = psv[:].rearrange("q (p h b) -> q p h b", p=2, h=2)
                        d4 = v2v[:, :, kt, :, 0:64]
                        nc.vector.tensor_copy(d4, s4)
                    return f

                for mt in (pair, 2 + pair):
                    steps.append(qk_step(mt))
                if pair == 1:
                    for kt in range(4 * qt, 4 * qt + 4):
                        steps.append(v_step(kt))
                return steps

            def S_steps(pair, qt):
                """One step per k-tile: paired QK matmuls (disjoint PE row
                groups) + one exp."""
                nk = 4 * (qt + 1)

                def kt_step(kt):
                    def f():
                        ps = psp.tile([128, 2 * TQ], f32, tag="S", bufs=2, name="psS")
                        m = kt - 4 * qt
                        off = max(0, m) * 128
                        for hh in range(2):
                            nc.tensor.matmul(
                                ps[:, hh * TQ + off:(hh + 1) * TQ],
                                lhsT=KTp[pair][hh * 64:(hh + 1) * 64,
                                               kt * 128:(kt + 1) * 128],
                                rhs=QTp[pair][hh * 64:(hh + 1) * 64,
                                              qt * TQ + off:(qt + 1) * TQ],
                                start=True, stop=True)
                        Pt = pP.tile([128, 2 * TQ], bf16, tag="P", bufs=32, name="Pt")
                        if m < 0:    # one exp across both heads' banks
                            nc.scalar.activation(Pt[:, :], ps[:, :], Exp, scale=0.125)
                        else:        # one strided exp covering both heads
                            pse = ps[:].rearrange("p (h w) -> p h w", h=2)[:, :, off:TQ]
                            pte = Pt[:].rearrange("p (h w) -> p h w", h=2)[:, :, off:TQ]
                            nc.scalar.activation(pte, pse, Exp, scale=0.125)
                            # diagonal block: keep k <= q only, both heads at once
                            ptm = Pt[:].rearrange("p (h w) -> p h w", h=2)[:, :, off:off + 128]
                            nc.vector.tensor_mul(
                                ptm, ptm,
                                tri2[:].rearrange("p (h w) -> p h w", h=2))
                        for hh in range(2):
                            h = 2 * pair + hh
                            P_store[(h, qt, kt)] = Pt[:, hh * TQ:(hh + 1) * TQ]
                    return f

                return [kt_step(kt) for kt in range(nk)]

            def PV_steps(pair, qt):
                """One step per head: PV accumulation (V stationary) plus the
                partition-broadcast normalization straight into yT2."""
                nk = 4 * (qt + 1)

                def head_step(hh):
                    def f():
                        h = 2 * pair + hh
                        psy = psp.tile([65, TQ], f32, tag="pv", bufs=2, name="psy")
                        for kt in range(nk):
                            off = max(0, kt - 4 * qt) * 128
                            nc.tensor.matmul(
                                psy[:, off:TQ],
                                lhsT=v2v[:, pair, kt, hh, 0:65],
                                rhs=P_store[(h, qt, kt)][:, off:TQ],
                                start=(kt == 0), stop=(kt == nk - 1))
                        drow = pRR.tile([1, TQ], f32, tag="rr", name="drow")
                        nc.vector.tensor_copy(drow[:], psy[64:65, :])
                        rec4 = pRR.tile([128, 4], f32, tag="r4", name="rec4")
                        nc.sync.dma_start(rec4[:, :], drow[0:1, :])
                        rec4b = pRR.tile([128, 4], bf16, tag="r4b", name="rec4b")
                        with nc.allow_low_precision("softmax recip in bf16"):
                            nc.vector.reciprocal(rec4b[:], rec4[:])
                        rrow = pRR.tile([1, TQ], bf16, tag="rrb", name="rrow")
                        nc.sync.dma_start(rrow[0:1, :], rec4b[:, :])
                        bcs = pBC.tile([64, TQ], bf16, tag="bc", name="bcs")
                        nc.gpsimd.partition_broadcast(bcs[:], rrow[0:1, :])
                        nc.vector.tensor_mul(
                            yT2[pair][hh * 64:(hh + 1) * 64, qt * TQ:(qt + 1) * TQ],
                            psy[0:64, :], bcs[:])
                        if hh == 1:
                            for kt in range(nk):
                                del P_store[(2 * pair, qt, kt)]
                                del P_store[(2 * pair + 1, qt, kt)]
                    return f

                return [head_step(0), head_step(1)]

            def co_mm(pso, pair, co, qt):
                nc.tensor.matmul(
                    pso[:],
                    lhsT=wp[:, pair * C + co * 128:pair * C + (co + 1) * 128],
                    rhs=yT2[pair][:, qt * TQ:(qt + 1) * TQ],
                    start=(pair == 0), stop=(pair == 1))

            def co_out(pso, co, qt):
                ob = pO.tile([128, TQ], bf16, tag="ob", name="ob")
                nc.vector.tensor_copy(ob[:], pso[:])
                nc.sync.dma_start(
                    out_d[co * 128:(co + 1) * 128, qt * TQ:(qt + 1) * TQ],
                    ob[:])

            def PJ_steps(qt):
                def co_step(co):
                    def f():
                        pso = psp.tile([128, TQ], f32, tag="big", bufs=2, name="psO")
                        for pair in range(2):
                            co_mm(pso, pair, co, qt)
                        co_out(pso, co, qt)
                    return f

                return [co_step(co) for co in range(8)]

            def PJ_steps_final(qt):
                """Final-strip projection: open pair-0 groups for the first
                four c_proj tiles early (they only need yT2[0], so they run
                while the last softmax-normalization chain is in flight),
                then close them and do the rest 4-deep across big+S psum."""
                pend = {}

                def open_step(co):
                    def f():
                        tag = "big" if co % 2 == 0 else "S"
                        pso = psp.tile([128, TQ], f32, tag=tag, bufs=2, name="psO")
                        co_mm(pso, 0, co, qt)
                        pend[co] = pso
                    return f

                def close_step(co):
                    def f():
                        pso = pend.pop(co)
                        co_mm(pso, 1, co, qt)
                        co_out(pso, co, qt)
                    return f

                def full_step(co):
                    def f():
                        tag = "big" if co % 2 == 0 else "S"
                        pso = psp.tile([128, TQ], f32, tag=tag, bufs=2, name="psO")
                        for pair in range(2):
                            co_mm(pso, pair, co, qt)
                        co_out(pso, co, qt)
                    return f

                return ([open_step(co) for co in range(4)]
                        + [close_step(co) for co in range(4)]
                        + [full_step(co) for co in range(4, 8)])

            def weave(s_list, others):
                """Interleave `others` proportionally between S k-tile steps."""
                if not s_list:
                    for f in others:
                        f()
                    return
                r = len(others) / len(s_list)
                acc, oi = 0.5, 0
                for f in s_list:
                    f()
                    acc += r
                    while acc >= 1.0 and oi < len(others):
                        others[oi]()
                        oi += 1
                        acc -= 1.0
                while oi < len(others):
                    others[oi]()
                    oi += 1

            # ---- software-pipelined, finely woven emission order -------
            for f in A_steps(0, 0):
                f()
            weave(S_steps(0, 0), A_steps(0, 1))
            weave(S_steps(0, 1), A_steps(1, 0))
            weave(S_steps(1, 0), PV_steps(0, 0) + A_steps(0, 2))
            weave(S_steps(0, 2), PV_steps(1, 0) + A_steps(1, 1) + PJ_steps(0))
            weave(S_steps(1, 1), PV_steps(0, 1) + A_steps(0, 3) + A_steps(1, 2))
            weave(S_steps(0, 3), PV_steps(1, 1) + A_steps(1, 3) + PJ_steps(1))
            weave(S_steps(1, 2), PV_steps(0, 2) + PV_steps(0, 3))
            weave(S_steps(1, 3), PV_steps(1, 2) + PJ_steps(2))
            weave([], PV_steps(1, 3) + PJ_steps_final(3))

    nc.compile()
    return nc


def _get_nc(with_bias: bool):
    key = ("nc", with_bias)
    if key not in _CACHE:
        _ensure_runtime()
        _CACHE[key] = _build(with_bias)
    return _CACHE[key]


def _shard_inputs(x, w_qkv, b_qkv, w_proj, with_bias):
    """Build the 8 per-core input maps (bf16), repacked for large DMA lines."""
    in_maps = []
    for core in range(N_CORES):
        b, g = core // G, core % G
        hs = [g * HPG + i for i in range(HPG)]
        q_cols = [w_qkv[:, h * DH:(h + 1) * DH] for h in hs]
        k_cols = [w_qkv[:, C + h * DH: C + (h + 1) * DH] for h in hs]
        v_cols = [w_qkv[:, 2 * C + h * DH: 2 * C + (h + 1) * DH] for h in hs]

        # x: [C, T] -> strip-major [strip*128, ci*512]
        xT = np.ascontiguousarray(x[b].T)              # [C, T]
        xs = xT.reshape(NCT, 128, NSTRIP, TQ)          # [ci, p, strip, col]
        xs = xs.transpose(2, 1, 0, 3).reshape(NSTRIP * 128, NCT * TQ)

        # wqk: [C, 512] (q01 q23 k01 k23) -> mt-major [mt*128, ci*128]
        wqk = np.concatenate(q_cols + k_cols, axis=1)  # [C, 512]
        ws = wqk.reshape(NCT, 128, 4, 128)             # [ci, p, mt, col]
        ws = ws.transpose(2, 1, 0, 3).reshape(4 * 128, NCT * 128)

        # wv: [C, 256] -> [128, ci*256]
        wv = np.concatenate(v_cols, axis=1)            # [C, 256]
        wvs = wv.reshape(NCT, 128, 256).transpose(1, 0, 2).reshape(128, NCT * 256)

        # wp: [256, C] -> [128, pair*1024]
        wpm = np.concatenate(
            [w_proj[h * DH:(h + 1) * DH, :] for h in hs], axis=0)  # [256, C]
        wps = wpm.reshape(2, 128, C).transpose(1, 0, 2).reshape(128, 2 * C)

        m = {
            "xT": np.ascontiguousarray(xs).astype(BF),
            "wqk": np.ascontiguousarray(ws).astype(BF),
            "wv": np.ascontiguousarray(wvs).astype(BF),
            "wp": np.ascontiguousarray(wps).astype(BF),
        }
        if with_bias:
            bq = [b_qkv[h * DH:(h + 1) * DH] for h in hs]
            bk = [b_qkv[C + h * DH: C + (h + 1) * DH] for h in hs]
            bvs = [b_qkv[2 * C + h * DH: 2 * C + (h + 1) * DH] for h in hs]
            m["bqk"] = np.concatenate(bq + bk)[None, :].astype(BF)
            m["bv"] = np.concatenate(bvs)[None, :].astype(BF)
        in_maps.append(m)
    return in_maps


def run_on_device(x, w_qkv, b_qkv, w_proj, b_proj, trace=False, trace_kwargs=None):
    """Returns (output [B,T,C] float32, BassKernelResults)."""
    x = np.asarray(x, np.float32)
    w_qkv = np.asarray(w_qkv, np.float32)
    b_qkv = np.asarray(b_qkv, np.float32)
    w_proj = np.asarray(w_proj, np.float32)
    b_proj = np.asarray(b_proj, np.float32)

    with_bias = bool(np.any(b_qkv))
    nc = _get_nc(with_bias)
    in_maps = _shard_inputs(x, w_qkv, b_qkv, w_proj, with_bias)

    from concourse.bass_utils import run_bass_kernel_spmd
    res = run_bass_kernel_spmd(nc, in_maps, core_ids=list(range(N_CORES)),
                               trace=trace, **(trace_kwargs or {}))

    out = np.zeros((B, T, C), np.float64)
    for core in range(N_CORES):
        b = core // G
        out[b] += res.results[core]["outT"].T.astype(np.float64)
    out += b_proj.astype(np.float64)[None, None, :]
    return out.astype(np.float32), res


def kernel(x, w_qkv, b_qkv, w_proj, b_proj):
    out, _ = run_on_device(x, w_qkv, b_qkv, w_proj, b_proj)
    return out
